# revision 1
# baseline (speedup 1.0000x reference)
"""Attention pooling kernel for Trainium2 (8 NeuronCores).

Computes: scores = E @ q; w = softmax(scores); out = w @ E
for E [N=2097152, 64] fp32, q [64] fp32.

Strategy (per core, N/8 = 262144 rows):
  - Host packs the core's row-shard into a "2-row-packed transposed" layout
    Epack [128, C=131072]: partition k = p*64 + d holds E[2n + p, d] at
    column n.  DMA is then contiguous per partition.
  - Scores via one TensorE matmul per 512-column chunk with a constant
    stationary operand qmat [128, 128], qmat[k, m] = q[k % 64] if
    (k // 64 == m // 64) else 0.  out[m, n] = s(2n + m//64): scores land
    REPLICATED across the 64 partitions of each parity half -> exp can use
    all 128 ACT lanes and the weights are already broadcast for the
    weighted-sum multiply.
  - ACT: w = exp(scores - C) PSUM->SBUF, fused accum_out gives per-chunk
    sumexp partials.
  - DVE: fused scalar_tensor_tensor (out = Epack * w, accum_out = per
    partition sum) accumulates the weighted sum: partition k = (p, d) gets
    sum_n E[2n+p, d] * w(2n+p).
  - Host: out[d] = (acc[d] + acc[64+d]) / (se[0] + se[64]), summed over
    cores.  The shift C (computed from q alone) cancels.
"""

import sys

sys.path.insert(0, "/opt/trn_rl_repo")

import numpy as np

N_TOTAL = 2097152
D = 64
N_CORES = 8
N_PER_CORE = N_TOTAL // N_CORES          # 262144
COLS_PER_CORE = N_PER_CORE // 2          # 131072 packed columns
MM_N = 512                               # matmul free dim (one PSUM bank)
DMA_COLS = 8192                          # columns per DMA tile
EXP_COLS = 1024                          # columns per exp op (2 PSUM banks)
MM_DT_NAME = "float32r"                  # scores matmul dtype; "float32" fallback
HWDGE_LANES = 1                          # DMA completion-sem lanes

_compiled = {}


def _build_nc(n_cols, dma_cols, mm_dt_name):
    import concourse.bacc as bacc
    import concourse.bass as bass
    import concourse.mybir as mybir
    import concourse.tile as tile

    fp32 = mybir.dt.float32
    bf16 = mybir.dt.bfloat16
    mm_dt = getattr(mybir.dt, mm_dt_name)

    nc = bacc.Bacc()
    ep_dram = nc.declare_dram_parameter("epack", [128, n_cols], mm_dt, isOutput=False)
    qmat_dram = nc.declare_dram_parameter("qmat", [128, 128], mm_dt, isOutput=False)
    cshift_dram = nc.declare_dram_parameter("cshift", [128, 1], fp32, isOutput=False)
    out_dram = nc.declare_dram_parameter("out", [128, 2], fp32, isOutput=True)

    n_tiles = n_cols // dma_cols

    with tile.TileContext(nc) as tc:
        with (
            tc.tile_pool(name="const", bufs=1) as const_pool,
            tc.tile_pool(name="ep", bufs=3) as ep_pool,
            tc.tile_pool(name="w", bufs=2) as w_pool,
            tc.tile_pool(name="junk", bufs=1) as junk_pool,
            tc.tile_pool(name="acc", bufs=1) as acc_pool,
            tc.tile_pool(name="se", bufs=4) as se_pool,
            tc.tile_pool(name="aw", bufs=3) as aw_pool,
            tc.tile_pool(name="psum", bufs=4, space=bass.MemorySpace.PSUM) as psum_pool,
        ):
            qmat = const_pool.tile([128, 128], mm_dt, tag="qmat")
            cshift = const_pool.tile([128, 1], fp32, tag="cshift")
            nc.sync.dma_start(qmat[:], qmat_dram[:])
            nc.sync.dma_start(cshift[:], cshift_dram[:])

            master_w = acc_pool.tile([128, 1], fp32, tag="master_w")
            master_se = acc_pool.tile([128, 1], fp32, tag="master_se")
            tmp_se = acc_pool.tile([128, 1], fp32, tag="tmp_se")

            groups = dma_cols // EXP_COLS
            for t in range(n_tiles):
                ep = ep_pool.tile([128, dma_cols], mm_dt, tag="ep")
                nc.sync.dma_start(ep[:], ep_dram[:, t * dma_cols:(t + 1) * dma_cols])

                w_sb = w_pool.tile([128, dma_cols], fp32, tag="w")
                accse = se_pool.tile([128, groups], fp32, tag="accse")
                for g in range(groups):
                    lo = g * EXP_COLS
                    ps = psum_pool.tile([128, EXP_COLS], fp32, tag="ps")
                    for k in range(EXP_COLS // MM_N):
                        nc.tensor.matmul(
                            ps[:, k * MM_N:(k + 1) * MM_N],
                            qmat[:],
                            ep[:, lo + k * MM_N:lo + (k + 1) * MM_N],
                            start=True,
                            stop=True,
                        )
                    # w = exp(scores - C); accum gives per-group sumexp
                    nc.scalar.activation(
                        w_sb[:, lo:lo + EXP_COLS],
                        ps[:],
                        mybir.ActivationFunctionType.Exp,
                        bias=cshift[:, 0:1],
                        scale=1.0,
                        accum_out=accse[:, g:g + 1],
                    )
                junk = junk_pool.tile([128, dma_cols], bf16, tag="junk")
                accw = aw_pool.tile([128, 1], fp32, tag="accw")
                nc.vector.scalar_tensor_tensor(
                    junk[:],
                    ep[:].bitcast(fp32),
                    1.0,
                    w_sb[:],
                    op0=mybir.AluOpType.mult,
                    op1=mybir.AluOpType.mult,
                    accum_out=accw[:],
                )
                if t == 0:
                    nc.vector.tensor_copy(master_w[:], accw[:])
                else:
                    nc.vector.tensor_add(master_w[:], master_w[:], accw[:])
                # fold sumexp partials (light DVE ops)
                nc.vector.tensor_reduce(
                    tmp_se[:], accse[:], axis=mybir.AxisListType.X,
                    op=mybir.AluOpType.add,
                )
                if t == 0:
                    nc.vector.tensor_copy(master_se[:], tmp_se[:])
                else:
                    nc.vector.tensor_add(master_se[:], master_se[:], tmp_se[:])

            res = acc_pool.tile([128, 2], fp32, tag="res")
            nc.vector.tensor_copy(res[:, 0:1], master_w[:])
            nc.vector.tensor_copy(res[:, 1:2], master_se[:])
            nc.sync.dma_start(out_dram[:], res[:])

    nc.compile()
    return nc


def _pack_core(e_core):
    # [Nc, 64] -> [n, p, d] -> [(p, d), n]
    nc_rows = e_core.shape[0]
    return np.ascontiguousarray(
        e_core.reshape(nc_rows // 2, 2, D).transpose(1, 2, 0).reshape(128, nc_rows // 2)
    )


def kernel(embeddings, query):
    from concourse.bass_utils import run_bass_kernel_spmd

    embeddings = np.asarray(embeddings, dtype=np.float32)
    query = np.asarray(query, dtype=np.float32)

    key = (COLS_PER_CORE, DMA_COLS, MM_DT_NAME)
    if key not in _compiled:
        _compiled[key] = _build_nc(*key)
    nc = _compiled[key]

    # constant shift for exp stability; cancels in the final division
    c_shift = float(6.0 * np.linalg.norm(query))

    qmat = np.zeros((128, 128), dtype=np.float32)
    qmat[0:64, 0:64] = query[:, None]      # qmat[k, m] = q[k] for m in first half
    qmat[64:128, 64:128] = query[:, None]
    cshift = np.full((128, 1), -c_shift, dtype=np.float32)

    in_maps = []
    for c in range(N_CORES):
        e_core = embeddings[c * N_PER_CORE:(c + 1) * N_PER_CORE]
        in_maps.append({
            "epack": _pack_core(e_core),
            "qmat": qmat,
            "cshift": cshift,
        })

    res = None
    for attempt in range(3):
        try:
            res = run_bass_kernel_spmd(nc, in_maps, list(range(N_CORES)))
            break
        except Exception:
            if attempt == 2:
                raise

    wsum = np.zeros(D, dtype=np.float64)
    sumexp = 0.0
    for r in res.results:
        out = r["out"].astype(np.float64)
        wsum += out[0:64, 0] + out[64:128, 0]
        sumexp += out[0, 1] + out[64, 1]
    return (wsum / sumexp).astype(np.float32)

